# revision 7
# baseline (speedup 1.0000x reference)
"""OHEM cross-entropy loss kernel for Trainium2 (8 NeuronCores, Bass/Tile). for Trainium2 (8 NeuronCores, Bass/Tile).

Math (matches reference.py):
    logp   = log_softmax(seg_logit, axis=1)          # [B,C,H,W], C=19
    x_l    = logp at label (ignore 255 -> class 0)
    prob   = exp(x_l)
    thr    = max(sort(prob.flatten())[MIN_KEPT*B], 0.7)
    loss   = mean(-x_l * (prob < thr))

Device strategy: data-parallel over B, one image per core.  Per core the
kernel returns partial sums of min(u,0) and counts of (u<0) where
u = x_l - lse - ln(0.7); the host combines them (valid when
count > MIN_KEPT*B, i.e. the 0.7 branch of the threshold; otherwise exact
host fallback).

Layout trick: the host sorts each image's pixels by label, so every
partition-row of F=256 pixels holds consecutive sorted pixels and is
label-pure except for the <=18 label-boundary rows per core.  The label
gather is then a per-partition-row DRAM row-gather: one indirect DMA per
chunk (SWDGE, runtime row indices from SBUF) fetches x_label rows straight
from DRAM, entirely off the compute engines.  Pixels in boundary rows
gather the row's first-pixel class; the host corrects those few pixels
exactly (it knows which they are).

Pipeline: 16 chunks of F=256 (one contiguous 2.5MB DMA each, 19KB
partition rows).  Per chunk: fused Exp (ACT) -> bf16, pairwise add tree
(DVE, 2x bf16) -> sumexp.  Every 2 chunks an incremental tail runs
(Ln + u + min/count accumulation) so almost nothing remains after the
last chunk.  acc [128, 2*NTAIL] f32 is the only output.
"""

import numpy as np

B = 8
C = 19
H, W = 512, 1024
HW = H * W            # 524288 pixels per image/core
P = 128               # SBUF partitions
F = 256               # pixels per partition per chunk
PF = P * F            # pixels per chunk
NCHUNK = HW // PF     # 16
HWP = NCHUNK * F      # pixels per partition per core (4096)
TGROUPS = [8, 7, 1]   # chunks per tail group (few Ln table loads, tiny final tail)
NTAIL = len(TGROUPS)
C0 = float(np.log(np.float32(0.7)))
MIN_KEPT = 100000
IGNORE_INDEX = 255
N_TOTAL = B * HW

_CACHE = {}


def _build_nc():
    import concourse.bacc as bacc
    import concourse.bass as bass
    import concourse.mybir as mybir
    import concourse.tile as tile

    fp32 = mybir.dt.float32
    bf16 = mybir.dt.bfloat16
    i32 = mybir.dt.int32

    nc = bacc.Bacc()
    xp = nc.dram_tensor("xp", [NCHUNK, P, C * F], bf16, kind="ExternalInput")
    cidx = nc.dram_tensor("cidx", [P, NCHUNK], i32, kind="ExternalInput")
    acc = nc.dram_tensor("acc", [P, 2 * NTAIL], fp32, kind="ExternalOutput")

    # row-table view for the gather: row (j, p, c) -> F contiguous floats
    xp_rows = xp[:, :, :].rearrange("j p (c f) -> (j p c) f", f=F)

    with tile.TileContext(nc) as tc:
        with (
            tc.tile_pool(name="lb", bufs=3) as lb_pool,
            tc.tile_pool(name="eb", bufs=2) as eb_pool,
            tc.tile_pool(name="idxp", bufs=1) as idx_pool,
            tc.tile_pool(name="allp", bufs=1) as all_pool,
            tc.tile_pool(name="tailp", bufs=2) as tail_pool,
            tc.tile_pool(name="accp", bufs=1) as acc_pool,
        ):
            cidx_t = idx_pool.tile([P, NCHUNK], i32)
            nc.sync.dma_start(out=cidx_t[:], in_=cidx[:, :])

            xl_all = all_pool.tile([P, HWP], bf16, tag="xl")
            se_all = all_pool.tile([P, HWP], fp32, tag="se")
            acc_t = acc_pool.tile([P, 2 * NTAIL], fp32)

            last_tree = None
            pen_tail_ops = []

            tail_end = np.cumsum(TGROUPS).tolist()  # [4, 8, 12, 15, 16]
            for j in range(NCHUNK):
                lb = lb_pool.tile([P, C * F], bf16, tag="lb")
                if j == 0:
                    # split first chunk 4-ways: first Exp starts much sooner
                    q = (C * F) // 4
                    for k in range(4):
                        dk = nc.sync.dma_start(
                            out=lb[:, k * q : (k + 1) * q],
                            in_=xp[0, :, k * q : (k + 1) * q],
                        )
                        if k == 0:
                            dma_i = dk
                else:
                    dma_i = nc.sync.dma_start(out=lb[:], in_=xp[j, :, :])

                # x_label row-gather straight from DRAM (off compute engines);
                # staggered behind this chunk's stream DMA so the gathers
                # don't pile up during ramp-in
                g_i = nc.gpsimd.indirect_dma_start(
                    out=xl_all[:, j * F : (j + 1) * F],
                    out_offset=None,
                    in_=xp_rows,
                    in_offset=bass.IndirectOffsetOnAxis(
                        ap=cidx_t[:, j : j + 1], axis=0
                    ),
                )
                tile.add_dep_helper(g_i, dma_i, reason="stagger gathers")

                eb = eb_pool.tile([P, C * F], bf16, tag="eb")
                if j == 0:
                    q = (C * F) // 4
                    for k in range(4):
                        nc.scalar.activation(
                            out=eb[:, k * q : (k + 1) * q],
                            in_=lb[:, k * q : (k + 1) * q],
                            func=mybir.ActivationFunctionType.Exp,
                        )
                else:
                    nc.scalar.activation(
                        out=eb[:], in_=lb[:],
                        func=mybir.ActivationFunctionType.Exp,
                    )

                # pairwise in-place add tree on eb (slots are contiguous F-runs)
                nc.vector.tensor_tensor(
                    out=eb[:, 0 : 9 * F], in0=eb[:, 0 : 9 * F],
                    in1=eb[:, 9 * F : 18 * F], op=mybir.AluOpType.add,
                )
                nc.vector.tensor_tensor(
                    out=eb[:, 0 : 4 * F], in0=eb[:, 0 : 4 * F],
                    in1=eb[:, 4 * F : 8 * F], op=mybir.AluOpType.add,
                )
                nc.vector.tensor_tensor(
                    out=eb[:, 8 * F : 9 * F], in0=eb[:, 8 * F : 9 * F],
                    in1=eb[:, 18 * F : 19 * F], op=mybir.AluOpType.add,
                )
                nc.vector.tensor_tensor(
                    out=eb[:, 0 : 2 * F], in0=eb[:, 0 : 2 * F],
                    in1=eb[:, 2 * F : 4 * F], op=mybir.AluOpType.add,
                )
                nc.vector.tensor_tensor(
                    out=eb[:, 0:F], in0=eb[:, 0:F], in1=eb[:, F : 2 * F],
                    op=mybir.AluOpType.add,
                )
                t6 = nc.vector.tensor_tensor(
                    out=se_all[:, j * F : (j + 1) * F], in0=eb[:, 0:F],
                    in1=eb[:, 8 * F : 9 * F], op=mybir.AluOpType.add,
                )
                if j == NCHUNK - 1:
                    last_tree = t6

                # incremental tail at group boundaries
                if (j + 1) in tail_end:
                    t = tail_end.index(j + 1)
                    g0 = 0 if t == 0 else tail_end[t - 1]
                    gn = (j + 1 - g0)
                    sl_ = slice(g0 * F, (j + 1) * F)
                    lse = tail_pool.tile([P, gn * F], fp32, tag="lse")
                    nc.scalar.activation(
                        out=lse[:], in_=se_all[:, sl_],
                        func=mybir.ActivationFunctionType.Ln,
                    )
                    u = tail_pool.tile([P, gn * F], fp32, tag="u")
                    o1 = nc.vector.scalar_tensor_tensor(
                        out=u[:], in0=xl_all[:, sl_], scalar=C0, in1=lse[:],
                        op0=mybir.AluOpType.subtract,
                        op1=mybir.AluOpType.subtract,
                    )
                    scr = tail_pool.tile([P, gn * F], fp32, tag="lse")
                    o2 = nc.vector.tensor_scalar(
                        out=scr[:], in0=u[:], scalar1=0.0, scalar2=None,
                        op0=mybir.AluOpType.min, op1=mybir.AluOpType.add,
                        accum_out=acc_t[:, t : t + 1],
                    )
                    scr2 = tail_pool.tile([P, gn * F], fp32, tag="lse")
                    o3 = nc.vector.tensor_scalar(
                        out=scr2[:], in0=u[:], scalar1=0.0, scalar2=None,
                        op0=mybir.AluOpType.is_lt, op1=mybir.AluOpType.add,
                        accum_out=acc_t[:, NTAIL + t : NTAIL + t + 1],
                    )
                    if t == NTAIL - 2:
                        pen_tail_ops = [o1, o2, o3]

            if last_tree is not None:
                for o in pen_tail_ops:
                    tile.add_dep_helper(
                        o.ins, last_tree.ins, sync=False,
                        reason="prioritize final tree",
                    )

            nc.sync.dma_start(out=acc[:, :], in_=acc_t[:])
    nc.finalize()
    return nc


def _host_fallback(seg_logit, seg_label):
    """Exact numpy replication of the reference (quantile path included)."""
    x = np.asarray(seg_logit, dtype=np.float32)
    lbl = np.asarray(seg_label)
    Bn, Cn = x.shape[0], x.shape[1]
    xf = x.reshape(Bn, Cn, -1)
    m = xf.max(axis=1, keepdims=True)
    e = np.exp(xf - m)
    lse = np.log(e.sum(axis=1, keepdims=True)) + m
    logp = xf - lse
    l2 = np.where(lbl == IGNORE_INDEX, 0, lbl).reshape(Bn, 1, -1).astype(np.int64)
    lp_at = np.take_along_axis(logp, l2, axis=1)[:, 0]
    prob = np.exp(lp_at)
    sortp = np.sort(prob.reshape(-1))
    idx = min(MIN_KEPT * Bn, sortp.shape[0] - 1)
    thr = max(float(sortp[idx]), np.float32(0.7))
    wgt = (prob < thr).astype(np.float32)
    return np.float32((-lp_at * wgt).mean())


def kernel(seg_logit, seg_label):
    from concourse import bass_utils

    x = np.asarray(seg_logit, dtype=np.float32).reshape(B, C, HW)
    lbl = np.asarray(seg_label).reshape(B, HW)
    lbl = np.where(lbl == IGNORE_INDEX, 0, lbl).astype(np.uint8)

    in_maps = []
    corr = []  # per-core host corrections for boundary-row pixels
    for b in range(B):
        perm = np.argsort(lbl[b], kind="stable")
        xd = np.take(x[b], perm, axis=1)  # [C, HW] sorted pixel order
        sl = lbl[b][perm]                 # sorted labels
        # chunk-major layout [NCHUNK, P, C, F]: partition row = F
        # consecutive sorted pixels, per-partition DRAM rows contiguous
        import ml_dtypes

        xpb = np.ascontiguousarray(
            xd.reshape(C, NCHUNK, P, F).transpose(1, 2, 0, 3)
        ).reshape(NCHUNK, P, C * F).astype(ml_dtypes.bfloat16)

        row_cls = sl[::F].astype(np.int64)  # [NCHUNK*P] first-pixel class
        jj = np.arange(NCHUNK * P) // P
        pp = np.arange(NCHUNK * P) % P
        cidx = np.zeros((P, NCHUNK), dtype=np.int32)
        cidx[pp, jj] = (jj * (P * C) + pp * C + row_cls).astype(np.int32)

        # boundary-row pixels where the row class is wrong: exact correction
        wrong = np.nonzero(sl != row_cls[np.arange(HW) // F])[0]
        if wrong.size:
            xw = xd[:, wrong]  # [C, n]
            m = xw.max(axis=0)
            lse = np.log(np.exp(xw - m).sum(axis=0)) + m
            u_r = xw[sl[wrong], np.arange(wrong.size)] - lse - C0
            u_w = xw[row_cls[wrong // F], np.arange(wrong.size)] - lse - C0
            d_racc = float(
                (np.minimum(u_r, 0) - np.minimum(u_w, 0)).sum(dtype=np.float64)
            )
            d_wacc = float((u_r < 0).sum() - (u_w < 0).sum())
            corr.append((d_racc, d_wacc))
        else:
            corr.append((0.0, 0.0))

        in_maps.append({"xp": xpb, "cidx": cidx})

    if "nc" not in _CACHE:
        _CACHE["nc"] = _build_nc()
    nc = _CACHE["nc"]

    res = bass_utils.run_bass_kernel_spmd(nc, in_maps, core_ids=list(range(B)))

    racc = 0.0
    wacc = 0.0
    for bi, r in enumerate(res.results):
        a = r["acc"]
        racc += float(a[:, :NTAIL].sum(dtype=np.float64)) + corr[bi][0]
        wacc += float(a[:, NTAIL:].sum(dtype=np.float64)) + corr[bi][1]

    if wacc <= MIN_KEPT * B:
        # quantile threshold exceeds 0.7 -> exact host path (rare/never for
        # the target distribution)
        return _host_fallback(seg_logit, seg_label)

    total = -(racc + C0 * wacc)
    return np.float32(total / N_TOTAL)


# revision 8
# speedup vs baseline: 1.0255x; 1.0255x over previous
"""OHEM cross-entropy loss kernel for Trainium2 (8 NeuronCores, Bass/Tile). for Trainium2 (8 NeuronCores, Bass/Tile).

Math (matches reference.py):
    logp   = log_softmax(seg_logit, axis=1)          # [B,C,H,W], C=19
    x_l    = logp at label (ignore 255 -> class 0)
    prob   = exp(x_l)
    thr    = max(sort(prob.flatten())[MIN_KEPT*B], 0.7)
    loss   = mean(-x_l * (prob < thr))

Device strategy: data-parallel over B, one image per core.  Per core the
kernel returns partial sums of min(u,0) and counts of (u<0) where
u = x_l - lse - ln(0.7); the host combines them (valid when
count > MIN_KEPT*B, i.e. the 0.7 branch of the threshold; otherwise exact
host fallback).

Layout trick: the host sorts each image's pixels by label, so every
partition-row of F=256 pixels holds consecutive sorted pixels and is
label-pure except for the <=18 label-boundary rows per core.  The label
gather is then a per-partition-row DRAM row-gather: one indirect DMA per
chunk (SWDGE, runtime row indices from SBUF) fetches x_label rows straight
from DRAM, entirely off the compute engines.  Pixels in boundary rows
gather the row's first-pixel class; the host corrects those few pixels
exactly (it knows which they are).

Pipeline: 16 chunks of F=256 (one contiguous 2.5MB DMA each, 19KB
partition rows).  Per chunk: fused Exp (ACT) -> bf16, pairwise add tree
(DVE, 2x bf16) -> sumexp.  Every 2 chunks an incremental tail runs
(Ln + u + min/count accumulation) so almost nothing remains after the
last chunk.  acc [128, 2*NTAIL] f32 is the only output.
"""

import numpy as np

B = 8
C = 19
H, W = 512, 1024
HW = H * W            # 524288 pixels per image/core
P = 128               # SBUF partitions
F = 256               # pixels per partition per chunk
PF = P * F            # pixels per chunk
NCHUNK = HW // PF     # 16
HWP = NCHUNK * F      # pixels per partition per core (4096)
TGROUPS = [4, 4, 4, 3, 1]   # chunks per tail group (small final tail)
NTAIL = len(TGROUPS)
C0 = float(np.log(np.float32(0.7)))
MIN_KEPT = 100000
IGNORE_INDEX = 255
N_TOTAL = B * HW

_CACHE = {}


def _build_nc():
    import concourse.bacc as bacc
    import concourse.bass as bass
    import concourse.mybir as mybir
    import concourse.tile as tile

    fp32 = mybir.dt.float32
    bf16 = mybir.dt.bfloat16
    i32 = mybir.dt.int32

    nc = bacc.Bacc()
    xp = nc.dram_tensor("xp", [NCHUNK, P, C * F], bf16, kind="ExternalInput")
    cidx = nc.dram_tensor("cidx", [P, NCHUNK], i32, kind="ExternalInput")
    acc = nc.dram_tensor("acc", [P, 2 * NTAIL], fp32, kind="ExternalOutput")

    # row-table view for the gather: row (j, p, c) -> F contiguous floats
    xp_rows = xp[:, :, :].rearrange("j p (c f) -> (j p c) f", f=F)

    with tile.TileContext(nc) as tc:
        with (
            tc.tile_pool(name="lb", bufs=3) as lb_pool,
            tc.tile_pool(name="eb", bufs=2) as eb_pool,
            tc.tile_pool(name="idxp", bufs=1) as idx_pool,
            tc.tile_pool(name="allp", bufs=1) as all_pool,
            tc.tile_pool(name="tailp", bufs=2) as tail_pool,
            tc.tile_pool(name="accp", bufs=1) as acc_pool,
        ):
            cidx_t = idx_pool.tile([P, NCHUNK], i32)
            nc.sync.dma_start(out=cidx_t[:], in_=cidx[:, :])

            xl_all = all_pool.tile([P, HWP], bf16, tag="xl")
            se_all = all_pool.tile([P, HWP], fp32, tag="se")
            acc_t = acc_pool.tile([P, 2 * NTAIL], fp32)

            last_tree = None
            pen_tail_ops = []

            tail_end = np.cumsum(TGROUPS).tolist()  # [4, 8, 12, 15, 16]
            for j in range(NCHUNK):
                lb = lb_pool.tile([P, C * F], bf16, tag="lb")
                if j == 0:
                    # split first chunk: halve the time to the first Exp
                    h = (C * F) // 2
                    dma_a = nc.sync.dma_start(
                        out=lb[:, :h], in_=xp[0, :, :h]
                    )
                    dma_i = nc.sync.dma_start(
                        out=lb[:, h:], in_=xp[0, :, h:]
                    )
                else:
                    dma_i = nc.sync.dma_start(out=lb[:], in_=xp[j, :, :])

                # x_label row-gather straight from DRAM (off compute engines);
                # staggered behind this chunk's stream DMA so the gathers
                # don't pile up during ramp-in
                g_i = nc.gpsimd.indirect_dma_start(
                    out=xl_all[:, j * F : (j + 1) * F],
                    out_offset=None,
                    in_=xp_rows,
                    in_offset=bass.IndirectOffsetOnAxis(
                        ap=cidx_t[:, j : j + 1], axis=0
                    ),
                )
                tile.add_dep_helper(g_i, dma_i, reason="stagger gathers")

                eb = eb_pool.tile([P, C * F], bf16, tag="eb")
                if j == 0:
                    h = (C * F) // 2
                    nc.scalar.activation(
                        out=eb[:, :h], in_=lb[:, :h],
                        func=mybir.ActivationFunctionType.Exp,
                    )
                    nc.scalar.activation(
                        out=eb[:, h:], in_=lb[:, h:],
                        func=mybir.ActivationFunctionType.Exp,
                    )
                else:
                    nc.scalar.activation(
                        out=eb[:], in_=lb[:],
                        func=mybir.ActivationFunctionType.Exp,
                    )

                # pairwise in-place add tree on eb (slots are contiguous F-runs)
                nc.vector.tensor_tensor(
                    out=eb[:, 0 : 9 * F], in0=eb[:, 0 : 9 * F],
                    in1=eb[:, 9 * F : 18 * F], op=mybir.AluOpType.add,
                )
                nc.vector.tensor_tensor(
                    out=eb[:, 0 : 4 * F], in0=eb[:, 0 : 4 * F],
                    in1=eb[:, 4 * F : 8 * F], op=mybir.AluOpType.add,
                )
                nc.vector.tensor_tensor(
                    out=eb[:, 8 * F : 9 * F], in0=eb[:, 8 * F : 9 * F],
                    in1=eb[:, 18 * F : 19 * F], op=mybir.AluOpType.add,
                )
                nc.vector.tensor_tensor(
                    out=eb[:, 0 : 2 * F], in0=eb[:, 0 : 2 * F],
                    in1=eb[:, 2 * F : 4 * F], op=mybir.AluOpType.add,
                )
                nc.vector.tensor_tensor(
                    out=eb[:, 0:F], in0=eb[:, 0:F], in1=eb[:, F : 2 * F],
                    op=mybir.AluOpType.add,
                )
                t6 = nc.vector.tensor_tensor(
                    out=se_all[:, j * F : (j + 1) * F], in0=eb[:, 0:F],
                    in1=eb[:, 8 * F : 9 * F], op=mybir.AluOpType.add,
                )
                if j == NCHUNK - 1:
                    last_tree = t6

                # incremental tail at group boundaries
                if (j + 1) in tail_end:
                    t = tail_end.index(j + 1)
                    g0 = 0 if t == 0 else tail_end[t - 1]
                    gn = (j + 1 - g0)
                    sl_ = slice(g0 * F, (j + 1) * F)
                    lse = tail_pool.tile([P, gn * F], fp32, tag="lse")
                    nc.scalar.activation(
                        out=lse[:], in_=se_all[:, sl_],
                        func=mybir.ActivationFunctionType.Ln,
                    )
                    u = tail_pool.tile([P, gn * F], fp32, tag="u")
                    o1 = nc.vector.scalar_tensor_tensor(
                        out=u[:], in0=xl_all[:, sl_], scalar=C0, in1=lse[:],
                        op0=mybir.AluOpType.subtract,
                        op1=mybir.AluOpType.subtract,
                    )
                    scr = tail_pool.tile([P, gn * F], fp32, tag="lse")
                    o2 = nc.vector.tensor_scalar(
                        out=scr[:], in0=u[:], scalar1=0.0, scalar2=None,
                        op0=mybir.AluOpType.min, op1=mybir.AluOpType.add,
                        accum_out=acc_t[:, t : t + 1],
                    )
                    scr2 = tail_pool.tile([P, gn * F], fp32, tag="lse")
                    o3 = nc.vector.tensor_scalar(
                        out=scr2[:], in0=u[:], scalar1=0.0, scalar2=None,
                        op0=mybir.AluOpType.is_lt, op1=mybir.AluOpType.add,
                        accum_out=acc_t[:, NTAIL + t : NTAIL + t + 1],
                    )
                    if t == NTAIL - 2:
                        pen_tail_ops = [o1, o2, o3]

            if last_tree is not None:
                for o in pen_tail_ops:
                    tile.add_dep_helper(
                        o.ins, last_tree.ins, sync=False,
                        reason="prioritize final tree",
                    )

            nc.sync.dma_start(out=acc[:, :], in_=acc_t[:])
    nc.finalize()
    return nc


def _host_fallback(seg_logit, seg_label):
    """Exact numpy replication of the reference (quantile path included)."""
    x = np.asarray(seg_logit, dtype=np.float32)
    lbl = np.asarray(seg_label)
    Bn, Cn = x.shape[0], x.shape[1]
    xf = x.reshape(Bn, Cn, -1)
    m = xf.max(axis=1, keepdims=True)
    e = np.exp(xf - m)
    lse = np.log(e.sum(axis=1, keepdims=True)) + m
    logp = xf - lse
    l2 = np.where(lbl == IGNORE_INDEX, 0, lbl).reshape(Bn, 1, -1).astype(np.int64)
    lp_at = np.take_along_axis(logp, l2, axis=1)[:, 0]
    prob = np.exp(lp_at)
    sortp = np.sort(prob.reshape(-1))
    idx = min(MIN_KEPT * Bn, sortp.shape[0] - 1)
    thr = max(float(sortp[idx]), np.float32(0.7))
    wgt = (prob < thr).astype(np.float32)
    return np.float32((-lp_at * wgt).mean())


def kernel(seg_logit, seg_label):
    from concourse import bass_utils

    x = np.asarray(seg_logit, dtype=np.float32).reshape(B, C, HW)
    lbl = np.asarray(seg_label).reshape(B, HW)
    lbl = np.where(lbl == IGNORE_INDEX, 0, lbl).astype(np.uint8)

    in_maps = []
    corr = []  # per-core host corrections for boundary-row pixels
    for b in range(B):
        perm = np.argsort(lbl[b], kind="stable")
        xd = np.take(x[b], perm, axis=1)  # [C, HW] sorted pixel order
        sl = lbl[b][perm]                 # sorted labels
        # chunk-major layout [NCHUNK, P, C, F]: partition row = F
        # consecutive sorted pixels, per-partition DRAM rows contiguous
        import ml_dtypes

        xpb = np.ascontiguousarray(
            xd.reshape(C, NCHUNK, P, F).transpose(1, 2, 0, 3)
        ).reshape(NCHUNK, P, C * F).astype(ml_dtypes.bfloat16)

        row_cls = sl[::F].astype(np.int64)  # [NCHUNK*P] first-pixel class
        jj = np.arange(NCHUNK * P) // P
        pp = np.arange(NCHUNK * P) % P
        cidx = np.zeros((P, NCHUNK), dtype=np.int32)
        cidx[pp, jj] = (jj * (P * C) + pp * C + row_cls).astype(np.int32)

        # boundary-row pixels where the row class is wrong: exact correction
        wrong = np.nonzero(sl != row_cls[np.arange(HW) // F])[0]
        if wrong.size:
            xw = xd[:, wrong]  # [C, n]
            m = xw.max(axis=0)
            lse = np.log(np.exp(xw - m).sum(axis=0)) + m
            u_r = xw[sl[wrong], np.arange(wrong.size)] - lse - C0
            u_w = xw[row_cls[wrong // F], np.arange(wrong.size)] - lse - C0
            d_racc = float(
                (np.minimum(u_r, 0) - np.minimum(u_w, 0)).sum(dtype=np.float64)
            )
            d_wacc = float((u_r < 0).sum() - (u_w < 0).sum())
            corr.append((d_racc, d_wacc))
        else:
            corr.append((0.0, 0.0))

        in_maps.append({"xp": xpb, "cidx": cidx})

    if "nc" not in _CACHE:
        _CACHE["nc"] = _build_nc()
    nc = _CACHE["nc"]

    res = bass_utils.run_bass_kernel_spmd(nc, in_maps, core_ids=list(range(B)))

    racc = 0.0
    wacc = 0.0
    for bi, r in enumerate(res.results):
        a = r["acc"]
        racc += float(a[:, :NTAIL].sum(dtype=np.float64)) + corr[bi][0]
        wacc += float(a[:, NTAIL:].sum(dtype=np.float64)) + corr[bi][1]

    if wacc <= MIN_KEPT * B:
        # quantile threshold exceeds 0.7 -> exact host path (rare/never for
        # the target distribution)
        return _host_fallback(seg_logit, seg_label)

    total = -(racc + C0 * wacc)
    return np.float32(total / N_TOTAL)
